# revision 87
# baseline (speedup 1.0000x reference)
"""Trainium2 Bass kernel for nn_AudioModelM1 (2x Mamba2 + selu + pool + heads).

Sharding: data-parallel over batch — 8 samples -> 8 NeuronCores, one sample per
core, no collectives.  Per-core layout is feature-major (features on SBUF
partitions, tokens on the free dim).  The selective scan uses the chunked
(quadratic-intra / recurrent-inter) Mamba2 formulation with Q=128 token chunks
so all heavy math runs on the TensorEngine.

Engine-balance notes (CoreSim cost model):
  - dt (softplus) is hoisted to a per-layer preamble and the RMSNorm scale +
    SELU are deferred to a per-layer epilogue so Ln never interleaves with the
    Exp/Tanh activations inside the block loop (act-table reloads are 1.3us).
  - silu(x) = x * (0.5*tanh(x/2) + 0.5): Tanh lives in the same activation
    table as Exp, so no table switches and no DVE reciprocal.
  - Small elementwise work in the scan inner loop runs on the Pool engine
    (flat 0.833 ns/elem, no access penalty); DVE ops keep all tensor operands
    packed bf16 in SBUF to hit the 2x/4x DVE perf modes.
  - Decay matrices are built per 4-head group: Pool stt folds (-cum_s + mask)
    so the Act exp runs batched over [Q, 4Q].
"""
import sys
sys.path.insert(0, "/opt/trn_rl_repo")

from contextlib import ExitStack

import numpy as np
import ml_dtypes

import concourse.bass as bass
import concourse.tile as tile
from concourse import bacc, mybir
from concourse.bass_utils import run_bass_kernel_spmd

FP32 = mybir.dt.float32
BF16 = mybir.dt.bfloat16
AL = mybir.AluOpType
AF = mybir.ActivationFunctionType

D = 1024
E = 2048
NST = 64
HD = 64
H = 32
DCONV = 4
CCH = E + 2 * NST             # 2176 conv channels (17 tiles)
F = 2 * E + 2 * NST + H       # 4256 in_proj rows
L = 2048
NCORE = 8

BLK = 256
NBLK = L // BLK
Q = 128
QPB = BLK // Q

KT_D = D // 128
MT_F = 34
CT = CCH // 128
ET = E // 128
HP = 4

SELU_L = 1.0507009873554805
SELU_A = 1.6732632423543772
SELU_LA = SELU_L * SELU_A
LN_LA = float(np.log(SELU_LA))
EPS = 1e-5

_CACHE = {}


def _bf(x):
    return np.ascontiguousarray(np.asarray(x, np.float32).astype(ml_dtypes.bfloat16))


def _f32(x):
    return np.ascontiguousarray(np.asarray(x, np.float32))


def _prep_layer(w, suf):
    in_w = np.asarray(w["in_proj_w" + suf], np.float32)
    out_w = np.asarray(w["out_proj_w" + suf], np.float32)
    norm_w = np.asarray(w["norm_w" + suf], np.float32)
    conv_w = np.asarray(w["conv_w" + suf], np.float32)
    conv_b = np.asarray(w["conv_b" + suf], np.float32)
    dt_b = np.asarray(w["dt_bias" + suf], np.float32)
    A_log = np.asarray(w["A_log" + suf], np.float32)
    Dp = np.asarray(w["D" + suf], np.float32)

    win = in_w.T.reshape(KT_D, 128, F).transpose(1, 0, 2)
    wo = (out_w * norm_w[None, :]).T
    wout = wo.reshape(ET, 128, D).transpose(1, 0, 2)
    cw = conv_w.reshape(CT, 128, DCONV).transpose(1, 0, 2)
    cb = conv_b.reshape(CT, 128).T
    dx = np.repeat(Dp, HD).reshape(ET, 128).T
    return {
        "win" + suf: _bf(win),
        "wout" + suf: _bf(wout),
        "cw" + suf: _f32(cw),
        "cb" + suf: _f32(cb),
        "dtb" + suf: _f32(dt_b.reshape(H, 1)),
        "A" + suf: _f32(-np.exp(A_log).reshape(H, 1)),
        "dx" + suf: _f32(dx),
    }


def _build():
    nc = bacc.Bacc("TRN2")
    dram = {}

    def din(name, shape, dt):
        dram[name] = nc.dram_tensor(name, list(shape), dt, kind="ExternalInput")
        return dram[name]

    xt = din("xt", (128, KT_D, L), BF16)
    for suf in ("1", "2"):
        din("win" + suf, (128, KT_D, F), BF16)
        din("wout" + suf, (128, ET, D), BF16)
        din("cw" + suf, (128, CT, DCONV), FP32)
        din("cb" + suf, (128, CT), FP32)
        din("dtb" + suf, (H, 1), FP32)
        din("A" + suf, (H, 1), FP32)
        din("dx" + suf, (128, ET), FP32)
    din("mask8", (128, 8 * Q), BF16)
    din("identb", (128, 128), BF16)
    din("identf", (128, 128), FP32)
    din("onesb", (128, 1), BF16)
    din("whead", (128, KT_D, 10), FP32)
    din("bcat", (1, 10), FP32)

    u2 = nc.dram_tensor("u2spill", [128, KT_D, L], BF16)
    ho_d = nc.dram_tensor("hospill", [128, KT_D, L], BF16)
    out_d = nc.dram_tensor("out", [1, 10], FP32, kind="ExternalOutput")

    with nc.allow_low_precision(reason="bf16 staging is intentional"), \
            tile.TileContext(nc) as tc, ExitStack() as ctx:
        pw = ctx.enter_context(tc.tile_pool(name="weights", bufs=1))
        pconst = ctx.enter_context(tc.tile_pool(name="consts", bufs=1))
        pio = ctx.enter_context(tc.tile_pool(name="io", bufs=2))
        pz = ctx.enter_context(tc.tile_pool(name="zsil", bufs=1))
        pxbc = ctx.enter_context(tc.tile_pool(name="xbcin", bufs=1))
        pxc = ctx.enter_context(tc.tile_pool(name="xconv", bufs=1))
        pg = ctx.enter_context(tc.tile_pool(name="gate", bufs=1))
        psc = ctx.enter_context(tc.tile_pool(name="scan", bufs=2))
        psc3 = ctx.enter_context(tc.tile_pool(name="scan3", bufs=3))
        pxt = ctx.enter_context(tc.tile_pool(name="xtok", bufs=1))
        pcm = ctx.enter_context(tc.tile_pool(name="chunkmeta", bufs=1))
        pb1 = ctx.enter_context(tc.tile_pool(name="bcq1", bufs=2))
        pb2 = ctx.enter_context(tc.tile_pool(name="bcq2", bufs=2))
        psm = ctx.enter_context(tc.tile_pool(name="small", bufs=2))
        pstate = ctx.enter_context(tc.tile_pool(name="state", bufs=1))
        pdt = ctx.enter_context(tc.tile_pool(name="dtpre", bufs=1))
        pcv1 = ctx.enter_context(tc.tile_pool(name="cv1", bufs=1))

        ps_mm = ctx.enter_context(tc.tile_pool(name="psmm", bufs=2, space="PSUM"))
        ps_tr = ctx.enter_context(tc.tile_pool(name="pstr", bufs=2, space="PSUM"))
        ps_yp = ctx.enter_context(tc.tile_pool(name="psyp", bufs=2, space="PSUM"))
        ps_sp = ctx.enter_context(tc.tile_pool(name="pssp", bufs=2, space="PSUM"))

        w_in = pw.tile([128, KT_D, F], BF16)
        w_out = pw.tile([128, ET, D], BF16)
        cw = pw.tile([128, CT, DCONV], FP32)
        cb = pw.tile([128, CT], FP32)
        dtb = pw.tile([H, 1], FP32)
        Atile = pw.tile([H, 1], FP32)
        dxt = pw.tile([128, ET], FP32)

        mask8 = pconst.tile([128, 8 * Q], BF16)
        identb = pconst.tile([128, 128], BF16)
        identf = pconst.tile([3 * H, 3 * H], FP32)
        onesb = pconst.tile([128, 1], BF16)
        whead = pconst.tile([128, KT_D, 10], FP32)
        bcat = pconst.tile([1, 10], FP32)
        zeros32 = pconst.tile([H, Q], FP32)
        ones32 = pconst.tile([H, 1], FP32)
        lnla_t = pconst.tile([128, 1], FP32)
        wdt2 = pconst.tile([128, KT_D, H], BF16)
        dtb2c = pconst.tile([H, 1], FP32)

        S = pstate.tile([NST, H * HD], BF16)   # [n, (h,p)] heads at base part 0
        pacc = pstate.tile([128, ET], FP32)

        # per-layer hoisted dt: raw -> exp -> (ln in place) -> dt; then alog
        dtf = pdt.tile([H, L], FP32)     # exp(raw+b) -> dt (f32) -> alog=dt*A
        lndttok = pdt.tile([Q, NBLK * QPB * H], FP32)  # token-major ln(dt)
        mall = pdt.tile([1, L], BF16)    # ssq/E+eps -> ln -> rmsnorm scale
        tailt = pdt.tile([128, CT, HP - 1], BF16)  # conv tail carry

        for t, name in ((mask8, "mask8"), (identb, "identb"),
                        (onesb, "onesb"), (whead, "whead"), (bcat, "bcat")):
            nc.sync.dma_start(t[:], dram[name][:])
        nc.sync.dma_start(identf[:], dram["identf"][0:3 * H, 0:3 * H])
        nc.sync.dma_start(wdt2[:], dram["win2"][:, :, F - H:F])
        nc.sync.dma_start(dtb2c[:], dram["dtb2"][:])
        nc.vector.memset(zeros32[:], 0.0)
        nc.vector.memset(pacc[:], 0.0)
        nc.vector.memset(ones32[:], 1.0)
        nc.vector.memset(lnla_t[:], LN_LA)

        for layer in (0, 1):
            suf = "12"[layer]
            for t, name in ((w_in, "win"), (w_out, "wout"), (cw, "cw"), (cb, "cb"),
                            (dtb, "dtb"), (Atile, "A"), (dxt, "dx")):
                nc.sync.dma_start(t[:], dram[name + suf][:])
            nc.vector.memset(S[:], 0.0)

            src = xt if layer == 0 else u2

            # ---- dt preamble: dt_raw for the whole layer, one softplus.
            # Layer 2's exp(raw+bias) is produced by the fused layer-1
            # epilogue, so only layer 1 runs the matmul sweep here.
            if layer == 0:
                for b in range(NBLK):
                    tsl = slice(b * BLK, (b + 1) * BLK)
                    u_t = pio.tile([128, KT_D, BLK], BF16, tag="ut")
                    nc.sync.dma_start(u_t[:], src[:, :, tsl])
                    pmmd = ps_mm.tile([H, BLK], FP32, tag="mm")
                    for kt in range(KT_D):
                        nc.tensor.matmul(pmmd[:], w_in[:, kt, F - H:F],
                                         u_t[:, kt, :], start=(kt == 0),
                                         stop=(kt == KT_D - 1))
                    nc.scalar.activation(dtf[:, tsl], pmmd[:], AF.Exp,
                                         bias=dtb[:])
            # softplus ln over the whole layer at once
            nc.scalar.activation(dtf[:], dtf[:], AF.Ln, bias=ones32[:])
            # token-major ln(dt) columns (still inside the Ln table window):
            # per-head decay matrices get dt folded in via the exp bias
            for c in range(NBLK * QPB):
                ptd = ps_tr.tile([Q, H], FP32, tag="tr")
                nc.tensor.transpose(ptd[:], dtf[:, c * Q:(c + 1) * Q],
                                    identf[0:H, 0:H])
                nc.scalar.activation(lndttok[:, c * H:(c + 1) * H], ptd[:],
                                     AF.Ln)
            nc.vector.tensor_scalar(dtf[:], dtf[:], Atile[:], None, op0=AL.mult)
            # dtf now holds alog = dt * A

            xbc = pxbc.tile([128, CT, BLK + HP], BF16)
            nc.vector.memset(xbc[:, :, 0:HP], 0.0)

            ut_store = {}

            def prep_inproj_xbc(b):
                """DMA u_t(b) now; return emit-closures for the xbc half of
                in_proj(b) to be woven into the previous block's scan."""
                u_t = pio.tile([128, KT_D, BLK], BF16, tag="ut")
                ut_store[b] = u_t
                nc.sync.dma_start(u_t[:], src[:, :, b * BLK:(b + 1) * BLK])
                steps = []
                if b > 0:
                    steps.append(lambda: nc.vector.tensor_copy(
                        tailt[:], xbc[:, :, BLK + 1:BLK + HP]))

                def mk(mt):
                    def go():
                        pmm = ps_mm.tile([128, BLK], FP32, tag="mm")
                        for kt in range(KT_D):
                            nc.tensor.matmul(
                                pmm[:], w_in[:, kt, mt * 128:(mt + 1) * 128],
                                u_t[:, kt, :], start=(kt == 0),
                                stop=(kt == KT_D - 1))
                        if mt % 2 == 0:
                            nc.vector.tensor_copy(
                                xbc[:, mt - ET, HP:HP + BLK], pmm[:])
                        else:
                            nc.scalar.copy(
                                xbc[:, mt - ET, HP:HP + BLK], pmm[:])
                    return go
                for mt in range(ET, 33):
                    steps.append(mk(mt))
                if b > 0:
                    steps.append(lambda: nc.vector.tensor_copy(
                        xbc[:, :, 1:HP], tailt[:]))
                return steps

            def emit_z(b, ho_iter=None):
                """z half of in_proj(b) -> sz; optionally interleave the
                previous block's out_proj groups from ho_iter."""
                u_t = ut_store.pop(b)
                sz = pz.tile([128, ET, BLK], BF16)
                sz_store[b] = sz
                for mt in range(ET):
                    if ho_iter is not None:
                        nxt = next(ho_iter, None)
                        if nxt is not None:
                            nxt()
                    pmm = ps_mm.tile([128, BLK], FP32, tag="mm")
                    for kt in range(KT_D):
                        nc.tensor.matmul(
                            pmm[:], w_in[:, kt, mt * 128:(mt + 1) * 128],
                            u_t[:, kt, :], start=(kt == 0), stop=(kt == KT_D - 1))
                    # silu(z) = z * (0.5*tanh(z/2) + 0.5)
                    th = psm.tile([128, BLK], BF16, tag="th")
                    nc.scalar.activation(th[:], pmm[:], AF.Tanh, scale=0.5)
                    zb = psm.tile([128, BLK], BF16, tag="zb")
                    nc.scalar.copy(zb[:], pmm[:])
                    nc.vector.tensor_scalar(th[:], th[:], 0.5, 0.5,
                                            op0=AL.mult, op1=AL.add)
                    nc.gpsimd.tensor_mul(sz[:, mt, :], th[:], zb[:])

            def emit_conv(b):
                # causal depthwise conv (+bias): 4 DVE tensor-scalar products
                # (4x perf mode) + 3 Pool tensor-tensor adds, then tanh-silu
                cv = pxc.tile([128, CT, BLK], BF16)
                cv_store[b] = cv
                for ct in range(CT):
                    cvs = cv[:, ct, :]
                    cvt = pcv1.tile([128, 3, BLK], BF16, tag="cvt")
                    nc.vector.tensor_scalar(cvs, xbc[:, ct, 1:1 + BLK],
                                            cw[:, ct, 0:1], cb[:, ct:ct + 1],
                                            op0=AL.mult, op1=AL.add)
                    for k in range(1, DCONV):
                        nc.vector.tensor_scalar(cvt[:, k - 1, :],
                                                xbc[:, ct, 1 + k:1 + k + BLK],
                                                cw[:, ct, k:k + 1], None,
                                                op0=AL.mult)
                    nc.gpsimd.tensor_add(cvt[:, 0, :], cvt[:, 0, :],
                                         cvt[:, 1, :])
                    nc.gpsimd.tensor_add(cvs, cvs, cvt[:, 2, :])
                    nc.gpsimd.tensor_add(cvs, cvs, cvt[:, 0, :])
                for cp in ((0, 4), (4, 4), (8, 4), (12, 4), (16, 1)):
                    c0, n_in = cp
                    w = n_in * BLK
                    cvp = cv[:, c0:c0 + n_in, :]
                    th2 = pcm.tile([128, 4 * BLK], BF16, tag="th2")
                    nc.scalar.activation(th2[:, 0:w], cvp, AF.Tanh, scale=0.5)
                    nc.vector.tensor_scalar(th2[:, 0:w], th2[:, 0:w], 0.5, 0.5,
                                            op0=AL.mult, op1=AL.add)
                    nc.gpsimd.tensor_tensor(cvp, th2[:, 0:w], cvp, op=AL.mult)
                # cv[:, 0:16] = x (silu'd), cv[:, 16] = B (parts 0:64) | C
                ctc = psm.tile([NST, BLK], BF16, tag="ctc")
                ctc_store[b] = ctc
                nc.sync.dma_start(ctc[:], cv[NST:128, 16, :])

            sz_store = {}
            cv_store = {}
            ctc_store = {}

            prologue = prep_inproj_xbc(0)
            for fn in prologue:
                fn()
            emit_conv(0)
            emit_z(0)

            for b in range(NBLK):
                tsl = slice(b * BLK, (b + 1) * BLK)
                cv = cv_store.pop(b)
                ctc = ctc_store.pop(b)
                steal = iter(prep_inproj_xbc(b + 1) if b + 1 < NBLK else [])

                g_sb = pg.tile([128, ET, BLK], BF16)

                for qi in range(QPB):
                    qsl = slice(qi * Q, (qi + 1) * Q)
                    gsl = slice(b * BLK + qi * Q, b * BLK + (qi + 1) * Q)
                    cidx = b * QPB + qi
                    cum = pcm.tile([H, Q], FP32, tag="cum")
                    nc.vector.tensor_tensor_scan(cum[:], dtf[:, gsl], zeros32[:],
                                                 0.0, op0=AL.add, op1=AL.add)
                    ptr = ps_tr.tile([Q, H], FP32, tag="tr")
                    nc.tensor.transpose(ptr[:], cum[:], identf[0:H, 0:H])
                    ctall = pcm.tile([Q, H], FP32, tag="ctall")
                    nc.scalar.copy(ctall[:], ptr[:])
                    # exp bias per head: ln(dt_s) - cum_s  (dt folded into lt)
                    negcl = pcm.tile([Q, H], FP32, tag="negcl")
                    nc.vector.tensor_sub(negcl[:],
                                         lndttok[:, cidx * H:(cidx + 1) * H],
                                         ctall[:])

                    ptb = ps_tr.tile([Q, NST], BF16, tag="tr")
                    nc.tensor.transpose(ptb[:], cv[0:NST, 16, qsl],
                                        identb[0:NST, 0:NST])
                    btok = pcm.tile([Q, NST], BF16, tag="btok")
                    nc.scalar.copy(btok[:], ptb[:])

                    g0 = ps_tr.tile([Q, Q], FP32, tag="tr")
                    nc.tensor.matmul(g0[:], cv[0:NST, 16, qsl], ctc[:, qsl])
                    g0sb = pcm.tile([Q, Q], BF16, tag="g0sb")
                    nc.scalar.copy(g0sb[:], g0[:])

                    xtok = pxt.tile([Q, E], BF16, tag="xtok")
                    for f4 in range(ET // 4):
                        ptx = ps_tr.tile([Q, 4 * 128], BF16, tag="tr")
                        for j in range(4):
                            nc.tensor.transpose(
                                ptx[:, j * 128:(j + 1) * 128],
                                cv[:, 4 * f4 + j, qsl], identb[:])
                        nc.vector.tensor_copy(
                            xtok[:, f4 * 512:(f4 + 1) * 512], ptx[:])

                    for _ in range(2):
                        fn = next(steal, None)
                        if fn is not None:
                            fn()
                    for hg in range(H // 8):
                        h0 = hg * 8
                        bcq8 = pb2.tile([Q, 8 * Q], FP32, tag="bcq")
                        for hh in range(2):
                            stg = psc.tile([1, 4 * Q], FP32, tag="stg")
                            nc.sync.dma_start(
                                stg[:], cum[h0 + 4 * hh:h0 + 4 * hh + 4, :])
                            nc.gpsimd.partition_broadcast(
                                bcq8[:, 4 * hh * Q:4 * (hh + 1) * Q], stg[:])
                        bce8 = pb1.tile([Q, 8 * Q], FP32, tag="bce")
                        nc.scalar.activation(bce8[:], bcq8[:], AF.Exp)
                        # mask AFTER bce8 snapshot (WAR dep keeps order)
                        nc.gpsimd.tensor_add(bcq8[:], bcq8[:], mask8[:])
                        for kp in range(4):
                            ftp = (h0 + 2 * kp) // 2
                            yp2 = ps_yp.tile([128, Q], FP32, tag="yp")
                            for k2 in range(2):
                                k = 2 * kp + k2
                                h = h0 + k
                                ro = k2 * 64
                                csl = slice(h * HD, (h + 1) * HD)
                                ksl = slice(k * Q, (k + 1) * Q)
                                lt = psc3.tile([Q, Q], FP32, tag="lt")
                                nc.scalar.activation(lt[:], bcq8[:, ksl],
                                                     AF.Exp,
                                                     bias=negcl[:, h:h + 1])
                                mt_t = psc3.tile([Q, Q], BF16, tag="mt")
                                nc.gpsimd.tensor_mul(mt_t[:], g0sb[:], lt[:])
                                cpos = psc3.tile([NST, Q], BF16, tag="cpos")
                                nc.gpsimd.tensor_mul(cpos[:], ctc[:, qsl],
                                                     bce8[0:NST, ksl])
                                bh = psc3.tile([Q, NST], BF16, tag="bh")
                                nc.vector.tensor_scalar(
                                    bh[:], btok[:],
                                    lt[:, Q - 1:Q],
                                    None, op0=AL.mult)
                                nc.tensor.matmul(yp2[ro:ro + 64, :],
                                                 xtok[:, csl], mt_t[:],
                                                 start=True, stop=False)
                                nc.tensor.matmul(yp2[ro:ro + 64, :],
                                                 S[:, csl], cpos[:],
                                                 start=False, stop=True)
                                sp = ps_sp.tile([NST, HD], FP32, tag="sp")
                                nc.tensor.matmul(sp[:], bh[:], xtok[:, csl])
                                nc.vector.scalar_tensor_tensor(
                                    S[:, csl], S[:, csl],
                                    bce8[0:NST, (k + 1) * Q - 1:(k + 1) * Q],
                                    sp[:], op0=AL.mult, op1=AL.add)
                            nc.vector.scalar_tensor_tensor(
                                g_sb[:, ftp, qsl], cv[:, ftp, qsl],
                                dxt[:, ftp:ftp + 1], yp2[:],
                                op0=AL.mult, op1=AL.add)
                        # weave a couple of next-block in_proj pieces into
                        # the scan so PE/DVE gaps get filled
                        for _ in range(2):
                            fn = next(steal, None)
                            if fn is not None:
                                fn()
                for fn in steal:
                    fn()

                # ---- gating, sum of squares (scale deferred)
                sz = sz_store.pop(b)
                for gq in range(4):
                    nc.gpsimd.tensor_tensor(g_sb[:, 4 * gq:4 * gq + 4, :],
                                            g_sb[:, 4 * gq:4 * gq + 4, :],
                                            sz[:, 4 * gq:4 * gq + 4, :],
                                            op=AL.mult)
                ssq = ps_mm.tile([1, BLK], FP32, tag="mm")
                for ft in range(ET):
                    g2 = psm.tile([128, BLK], BF16, tag="th")
                    nc.vector.tensor_tensor(g2[:], g_sb[:, ft, :],
                                            g_sb[:, ft, :], op=AL.mult)
                    nc.tensor.matmul(ssq[:], onesb[:], g2[:],
                                     start=(ft == 0), stop=(ft == ET - 1))
                nc.vector.tensor_scalar(mall[0:1, tsl], ssq[:], 1.0 / E, EPS,
                                        op0=AL.mult, op1=AL.add)

                # conv of the next block runs on DVE/Pool while out_proj
                # owns the TensorEngine
                if b + 1 < NBLK:
                    emit_conv(b + 1)

                def mk_ho(mt):
                    def go():
                        ho = ps_mm.tile([128, BLK], FP32, tag="mm")
                        for kt in range(ET):
                            nc.tensor.matmul(
                                ho[:], w_out[:, kt, mt * 128:(mt + 1) * 128],
                                g_sb[:, kt, :],
                                start=(kt == 0), stop=(kt == ET - 1))
                        hob = psm.tile([128, BLK], BF16, tag="hob")
                        if mt % 2 == 0:
                            nc.vector.tensor_copy(hob[:], ho[:])
                        else:
                            nc.scalar.copy(hob[:], ho[:])
                        nc.sync.dma_start(ho_d[:, mt, tsl], hob[:])
                    return go
                ho_steps = iter([mk_ho(mt) for mt in range(ET // 2)])
                if b + 1 < NBLK:
                    emit_z(b + 1, ho_iter=ho_steps)
                for fn in ho_steps:
                    fn()

            # ---- layer epilogue: rmsnorm scale + selu (+ spill / pooling)
            nc.scalar.activation(mall[:], mall[:], AF.Ln)
            nc.scalar.activation(mall[:], mall[:], AF.Exp, scale=-0.5)
            for b in range(NBLK):
                tsl = slice(b * BLK, (b + 1) * BLK)
                scb = psm.tile([128, BLK], BF16, tag="scb")
                nc.gpsimd.partition_broadcast(scb[:], mall[0:1, tsl])
                if layer == 0:
                    u2blk = pio.tile([128, KT_D, BLK], BF16, tag="ut")
                for mt in range(0, ET // 2, 2):
                    hot = pio.tile([128, 2, BLK], BF16, tag="hot")
                    nc.sync.dma_start(hot[:], ho_d[:, mt:mt + 2, tsl])
                    nc.gpsimd.tensor_mul(hot[:, 0, :], hot[:, 0, :], scb[:])
                    nc.gpsimd.tensor_mul(hot[:, 1, :], hot[:, 1, :], scb[:])
                    rl = psm.tile([128, 2, BLK], BF16, tag="rl")
                    nc.scalar.activation(rl[:], hot[:], AF.Relu, scale=SELU_L)
                    ex = psm.tile([128, 2, BLK], BF16, tag="ex")
                    nc.scalar.activation(ex[:], hot[:], AF.Exp, bias=lnla_t[:])
                    nc.vector.tensor_scalar(ex[:], ex[:], SELU_LA, SELU_LA,
                                            op0=AL.min, op1=AL.subtract)
                    if layer == 0:
                        nc.gpsimd.tensor_tensor(u2blk[:, mt:mt + 2, :], rl[:],
                                                ex[:], op=AL.add)
                    else:
                        nc.gpsimd.tensor_tensor(rl[:], rl[:], ex[:], op=AL.add)
                        for j in range(2):
                            red = psm.tile([128, 1], FP32, tag="red")
                            nc.vector.tensor_reduce(red[:], rl[:, j, :],
                                                    axis=mybir.AxisListType.X,
                                                    op=AL.add)
                            nc.vector.tensor_add(pacc[:, mt + j:mt + j + 1],
                                                 pacc[:, mt + j:mt + j + 1],
                                                 red[:])
                if layer == 0:
                    nc.sync.dma_start(u2[:, :, tsl], u2blk[:])
                    pmmd = ps_mm.tile([H, BLK], FP32, tag="mm")
                    for kt in range(KT_D):
                        nc.tensor.matmul(pmmd[:], wdt2[:, kt, :],
                                         u2blk[:, kt, :], start=(kt == 0),
                                         stop=(kt == KT_D - 1))
                    nc.scalar.activation(dtf[:, tsl], pmmd[:], AF.Exp,
                                         bias=dtb2c[:])

        pooled = psm.tile([128, KT_D], FP32, tag="pooled")
        nc.vector.tensor_scalar(pooled[:], pacc[:, 0:KT_D], 1.0 / L, None,
                                op0=AL.mult)
        ph = ps_sp.tile([1, 10], FP32, tag="sp")
        for kt in range(KT_D):
            nc.tensor.matmul(ph[:], pooled[:, kt:kt + 1], whead[:, kt, :],
                             start=(kt == 0), stop=(kt == KT_D - 1))
        ot = psm.tile([1, 10], FP32, tag="ot")
        nc.vector.tensor_add(ot[:], ph[:], bcat[:])
        nc.sync.dma_start(out_d[:], ot[:])

    nc.compile()
    return nc


def _host_inputs(inputs):
    m = {}
    m.update(_prep_layer(inputs, "1"))
    m.update(_prep_layer(inputs, "2"))
    j = np.arange(128)
    mneg = np.where(j[None, :] >= j[:, None], 0.0, -1e30)
    m["mask8"] = _bf(np.tile(mneg, (1, 8)))
    m["identb"] = _bf(np.eye(128))
    m["identf"] = _f32(np.eye(128))
    m["onesb"] = _bf(np.ones((128, 1)))
    wcat = np.concatenate([np.asarray(inputs["w_emo"], np.float32),
                           np.asarray(inputs["w_sen"], np.float32)], 0)
    m["whead"] = _f32(wcat.T.reshape(KT_D, 128, 10).transpose(1, 0, 2))
    m["bcat"] = _f32(np.concatenate([inputs["b_emo"], inputs["b_sen"]])
                     .reshape(1, 10))
    return m


def kernel(**inputs) -> np.ndarray:
    if "nc" not in _CACHE:
        _CACHE["nc"] = _build()
    nc = _CACHE["nc"]

    x = np.asarray(inputs["x"], np.float32)
    shared = _host_inputs(inputs)
    in_maps = []
    for s in range(NCORE):
        m = dict(shared)
        xts = x[s].T.reshape(KT_D, 128, L).transpose(1, 0, 2)
        m["xt"] = _bf(xts)
        in_maps.append(m)

    res = run_bass_kernel_spmd(nc, in_maps, core_ids=list(range(NCORE)))
    out = np.concatenate([r["out"] for r in res.results], 0)
    return out.astype(np.float32)
